# revision 28
# baseline (speedup 1.0000x reference)
"""AdaptiveGCNLayer on 8 TRN2 NeuronCores (Bass/Tile), self-contained. v4.

Math (algebraically reduced from the reference):
    deg[i]  = 1 + indegree_col(i);  dis = 1/sqrt(deg)
    P[c]    = dis[c] * ( sum_{e:(r->c)} dis[r]*x[r]  +  dis[c]*x[c] )
    R[r]    = sum_{e:(r->c)} x[c];   Q = x * R
    h_align = P @ W_amp + b_amp
    h_div   = relu(P @ W_dmp + b_dmp) + Q @ W_diff + cnt_row[:,None]*b_diff
    alpha   = sigmoid(relu([h_align|h_div] @ Wg1 + bg1) @ Wg2 + bg2)
    out     = alpha*h_align + (1-alpha)*h_div

v4 PAIRED GATHER: HW gather cost is ~2.2ns PER DESCRIPTOR regardless of
payload size (512B costs the same as 256B), so per core+pass we build a
custom table whose 512B rows hold TWO source vectors [x[u] | x[v]] for
edge pairs (u->*, v->*) targeting the SAME dest block. One descriptor then
feeds two edges; descriptors halve (~50k vs ~100k per pass). Each 128-slot
chunk scatters via TWO one-hot matmuls (halves A and B; dead halves use
off=200 which never matches the iota compare). Rows are allocated so one
block's rows live in a single region (no straddle), keeping (block,region)
group fragmentation down. GCAP=4 (512-idx calls, 33 ring slots) keeps 3
calls in flight per queue of the fixed ~128-slot ucode ring.

Known HW walls: gather calls >1024 idxs hang (fixed ucode ring);
duplicate-row gathers are ~3x slower (pad slots use distinct consecutive
idxs); descriptor cost is flat 256B->512B but NOT beyond.

Measured dead ends (do not retry): QUAD rows (1KB, 4 sources/desc,
25k descs/pass) ran 417us vs 403us — beyond 512B the DMA pays per byte
and desc count is no longer the wall. SBW=4 + psacc bufs=2 (PSUM
double-buffering of adjacent superblocks) ran 453us — doubled per-sb
merge/dense/slice overheads outweigh the overlap. Remaining gap vs the
~250us engine floors is a ~1us/call Pool overhead beyond the
994+0.34/idx model plus the S-build chain on DVE (~108 elem/ns with
broadcast operands); candidate next steps: merge P/R scatter into one
interleaved-PSUM pass, or move S one-hot generation off DVE.
"""
import sys

if "/opt/trn_rl_repo" not in sys.path:
    sys.path.insert(0, "/opt/trn_rl_repo")

import numpy as np
import ml_dtypes

N_NODES = 100000
F = 128
F2 = 256                       # paired row: two 128-f vectors
N_CORES = 8
SH = N_NODES // N_CORES        # 12500 nodes per core
REG = 4                        # gather source regions (int16 index limit)
PREG = 12500                   # pair-rows per region
PREGPAD = PREG + 1
NPAIRS = REG * PREG            # 50000 table rows
W = 128                        # dest block width == one-hot width
NB = (SH + W - 1) // W         # 98 blocks
NBW = NB * W                   # padded dest width 12544
SBW = 8                        # blocks per superblock (8*128 dests = 2 PSUM banks)
NSB = (NB + SBW - 1) // SBW    # 13 superblocks
CB = 8                         # chunks per S-build DVE op
GCAP = 8                       # whole-group calls, shortened to 1008 idxs = 64 ring
                               # slots so TWO fit in the ~128-slot per-queue ring
DMA_SCRATCH = 16384

bf16 = ml_dtypes.bfloat16

_CACHE = {}
_LAST_EXEC_NS = None


# ----------------------------------------------------------------------------
# host-side planning
# ----------------------------------------------------------------------------

def _pair_stream(u, dest_local):
    """One core+pass: pair edge occurrences within each dest block.
    Returns (rows_u, rows_v, row_block, offA, offB) — one slot per row."""
    blk = dest_local // W
    off = dest_local % W
    order = np.argsort(blk, kind="stable")
    ub, ob, bb = u[order], off[order], blk[order]
    bnd = np.searchsorted(bb, np.arange(NB + 1))
    RU, RV, RB, RA, RO = [], [], [], [], []
    for b in range(NB):
        s, e = int(bnd[b]), int(bnd[b + 1])
        n = e - s
        if n == 0:
            continue
        m = n // 2
        if m:
            RU.append(ub[s:s + 2 * m:2]);  RV.append(ub[s + 1:s + 2 * m:2])
            RA.append(ob[s:s + 2 * m:2]);  RO.append(ob[s + 1:s + 2 * m:2])
            RB.append(np.full(m, b, np.int64))
        if n % 2:
            RU.append(ub[e - 1:e]); RV.append(ub[e - 1:e])
            RA.append(ob[e - 1:e]); RO.append(np.full(1, 200, np.int64))
            RB.append(np.full(1, b, np.int64))
    cat = lambda L: np.concatenate(L) if L else np.zeros(0, np.int64)
    return cat(RU), cat(RV), cat(RB), cat(RA), cat(RO)


def _assign_regions(row_block):
    """Allocate each block's rows into region block%REG (SHARED across cores
    so the max-over-cores plan stays dense; positions are per-core)."""
    cur = [r * PREG for r in range(REG)]
    lim = [(r + 1) * PREG for r in range(REG)]
    row_j = np.empty(len(row_block), np.int64)
    i = 0
    n_all = len(row_block)
    while i < n_all:
        b = int(row_block[i])
        j = i
        while j < n_all and row_block[j] == b:
            j += 1
        n = j - i
        r = b % REG
        assert lim[r] - cur[r] >= n, "region overflow"
        row_j[i:j] = cur[r] + np.arange(n)
        cur[r] += n
        i = j
    return row_j


def _plan(dest_b, reg_arr):
    """Shared chunk layout: nch[b, r] = chunks for (block b, region r), max over cores."""
    counts = np.zeros((N_CORES, NB, REG), np.int64)
    for k in range(N_CORES):
        g = dest_b[k] * REG + reg_arr[k]
        counts[k] = np.bincount(g, minlength=NB * REG).reshape(NB, REG)
    # +16: reserve >=16 pad slots per segment tail so each group's final
    # gather call can be shortened to 1008 idxs (64 ring slots -> 2/queue)
    nch = -(-(counts.max(axis=0) + 16) // 128)
    empty = nch.sum(axis=1) == 0
    nch[empty, 0] = 1
    return nch


def _group_meta(nch):
    """Stream-ordered groups (sb, r, b) with sizes/bases + per-chunk metadata."""
    gid_order = []
    gsizes = []
    for sb in range(NSB):
        for r in range(REG):
            for b in range(sb * SBW, min((sb + 1) * SBW, NB)):
                gid_order.append((b, r))
                gsizes.append(int(nch[b, r]) * 128)
    gbase = np.zeros(len(gsizes) + 1, np.int64)
    gbase[1:] = np.cumsum(gsizes)
    gpos = np.full((NB, REG), -1, np.int64)
    for i, (b, r) in enumerate(gid_order):
        gpos[b, r] = i

    nbank = (NB + 3) // 4
    total_by_bank = np.zeros(nbank, np.int64)
    for b in range(NB):
        total_by_bank[b // 4] += nch[b].sum()
    chunk_meta = []   # (sb, r, block_local, start, stop), stream order
    group_meta = []   # (sb, r, n_chunks, base_slot)
    seen = np.zeros(nbank, np.int64)
    gi = 0
    for sb in range(NSB):
        for r in range(REG):
            blocks = range(sb * SBW, min((sb + 1) * SBW, NB))
            n_in_group = sum(int(nch[b, r]) for b in blocks)
            group_meta.append((sb, r, n_in_group, int(gbase[gi]) if n_in_group else -1))
            for b in blocks:
                bk = b // 4
                for _ in range(int(nch[b, r])):
                    chunk_meta.append((sb, r, b - sb * SBW,
                                       seen[bk] == 0, seen[bk] == total_by_bank[bk] - 1))
                    seen[bk] += 1
                gi += 1
    return gpos, gbase, chunk_meta, group_meta


def _stream(s_blk, s_reg, s_idx, s_offA, s_offB, gpos, gbase):
    """Per-core padded slot stream -> (idx, offA, offB) full arrays."""
    total = int(gbase[-1])
    gi = gpos[s_blk, s_reg]
    order = np.lexsort((s_idx, gi))
    gi_s = gi[order]
    first = np.searchsorted(gi_s, gi_s)
    slots = gbase[gi_s] + (np.arange(len(order)) - first)
    # pad slots: distinct consecutive idxs (dup-row gathers are ~3x slower on
    # HW) neutralized via off=200 (never matches iota 0..127 -> zero S row)
    idxs = (np.arange(total) % PREG).astype(np.int32)
    offA = np.full(total, 200, np.int64)
    offB = np.full(total, 200, np.int64)
    idxs[slots] = s_idx[order]
    offA[slots] = s_offA[order]
    offB[slots] = s_offB[order]
    return idxs, offA, offB


def _pair_table(halves, rows_u, rows_v, row_j):
    flat = np.zeros((NPAIRS, F2), bf16)
    flat[row_j, :F] = halves[rows_u]
    flat[row_j, F:] = halves[rows_v]
    outp = np.zeros((REG * PREGPAD, F2), bf16)
    for r in range(REG):
        outp[r * PREGPAD:r * PREGPAD + PREG] = flat[r * PREG:(r + 1) * PREG]
    return outp


def _wrap_idx(idxs):
    w = idxs.reshape(-1, 16).T.astype(np.int16)
    return np.ascontiguousarray(np.tile(w, (8, 1)))


def _wrap_off(offs):
    return np.ascontiguousarray(offs.reshape(-1, 128).T.astype(bf16))


def _call_sizes(n):
    if n <= 0:
        return []
    k = -(-n // GCAP)
    lo = n // k
    hi_cnt = n - lo * k
    return [lo + 1] * hi_cnt + [lo] * (k - hi_cnt)


# ----------------------------------------------------------------------------
# graph builder
# ----------------------------------------------------------------------------

def _build_graph(cmP, gmP, cmR, gmR, lenP, lenR):
    import concourse.bass as bass  # noqa: F401
    from concourse import bacc
    import concourse.mybir as mybir
    import concourse.tile as tile

    dt = mybir.dt
    AF = mybir.ActivationFunctionType
    CP = lenP // 128
    CR = lenR // 128

    nc = bacc.Bacc(None, target_bir_lowering=False, num_swdge_queues=4,
                   dynamic_dma_scratch_size=DMA_SCRATCH)

    xdis_d = nc.declare_dram_parameter("xdis", [REG * PREGPAD, F2], dt.bfloat16, isOutput=False)
    xraw_d = nc.declare_dram_parameter("xraw", [REG * PREGPAD, F2], dt.bfloat16, isOutput=False)
    idxP_d = nc.declare_dram_parameter("idxP", [128, lenP // 16], dt.int16, isOutput=False)
    idxR_d = nc.declare_dram_parameter("idxR", [128, lenR // 16], dt.int16, isOutput=False)
    offPA_d = nc.declare_dram_parameter("offPA", [128, CP], dt.bfloat16, isOutput=False)
    offPB_d = nc.declare_dram_parameter("offPB", [128, CP], dt.bfloat16, isOutput=False)
    offRA_d = nc.declare_dram_parameter("offRA", [128, CR], dt.bfloat16, isOutput=False)
    offRB_d = nc.declare_dram_parameter("offRB", [128, CR], dt.bfloat16, isOutput=False)
    xlocT_d = nc.declare_dram_parameter("xlocT", [128, NBW], dt.bfloat16, isOutput=False)
    xdisT_d = nc.declare_dram_parameter("xdisT", [128, NBW], dt.bfloat16, isOutput=False)
    dcnt_d = nc.declare_dram_parameter("dcnt", [2, NBW], dt.bfloat16, isOutput=False)
    iota_d = nc.declare_dram_parameter("iota", [128, CB * W], dt.bfloat16, isOutput=False)
    wamp_d = nc.declare_dram_parameter("wamp", [128, 128], dt.bfloat16, isOutput=False)
    wdmp_d = nc.declare_dram_parameter("wdmp", [128, 128], dt.bfloat16, isOutput=False)
    wdiff_d = nc.declare_dram_parameter("wdiff", [128, 128], dt.bfloat16, isOutput=False)
    wg1a_d = nc.declare_dram_parameter("wg1a", [128, 128], dt.bfloat16, isOutput=False)
    wg1b_d = nc.declare_dram_parameter("wg1b", [128, 128], dt.bfloat16, isOutput=False)
    wg2_d = nc.declare_dram_parameter("wg2", [128, 1], dt.bfloat16, isOutput=False)
    bdiff_d = nc.declare_dram_parameter("bdiffT", [1, 128], dt.bfloat16, isOutput=False)
    ones_d = nc.declare_dram_parameter("ones1", [1, 128], dt.bfloat16, isOutput=False)
    bamp_d = nc.declare_dram_parameter("bamp", [128, 1], dt.float32, isOutput=False)
    bdmp_d = nc.declare_dram_parameter("bdmp", [128, 1], dt.float32, isOutput=False)
    bg1_d = nc.declare_dram_parameter("bg1", [128, 1], dt.float32, isOutput=False)
    bg2_d = nc.declare_dram_parameter("bg2", [1, 1], dt.float32, isOutput=False)
    out_d = nc.declare_dram_parameter("out", [128, NBW], dt.bfloat16, isOutput=True)

    with tile.TileContext(nc) as tc:
        with tc.tile_pool(name="persist", bufs=1) as pp:
            iota_t = pp.tile([128, CB * W], dt.bfloat16)
            offPA_t = pp.tile([128, CP], dt.bfloat16)
            offPB_t = pp.tile([128, CP], dt.bfloat16)
            offRA_t = pp.tile([128, CR], dt.bfloat16)
            offRB_t = pp.tile([128, CR], dt.bfloat16)
            idxP_t = pp.tile([128, lenP // 16], dt.int16)
            idxR_t = pp.tile([128, lenR // 16], dt.int16)
            wamp = pp.tile([128, 128], dt.bfloat16)
            wdmp = pp.tile([128, 128], dt.bfloat16)
            wdiff = pp.tile([128, 128], dt.bfloat16)
            wg1a = pp.tile([128, 128], dt.bfloat16)
            wg1b = pp.tile([128, 128], dt.bfloat16)
            wg2 = pp.tile([128, 1], dt.bfloat16)
            bdiffT = pp.tile([1, 128], dt.bfloat16)
            ones1 = pp.tile([1, 128], dt.bfloat16)
            bamp = pp.tile([128, 1], dt.float32)
            bdmp = pp.tile([128, 1], dt.float32)
            bg1 = pp.tile([128, 1], dt.float32)
            bg2 = pp.tile([1, 1], dt.float32)

            cut0 = gmP[0][2] * 128 // 16
            cutP = sum(n * 128 for (sb, _, n, _) in gmP if sb == 0) // 16
            cutR = sum(n * 128 for (sb, _, n, _) in gmR if sb == 0) // 16
            nc.sync.dma_start(out=idxP_t[:, :cut0], in_=idxP_d[:, :cut0])
            nc.sync.dma_start(out=idxP_t[:, cut0:cutP], in_=idxP_d[:, cut0:cutP])
            nc.sync.dma_start(out=idxR_t[:, :cutR], in_=idxR_d[:, :cutR])
            for t_, d_ in [(iota_t, iota_d), (offPA_t, offPA_d), (offPB_t, offPB_d),
                           (offRA_t, offRA_d), (offRB_t, offRB_d),
                           (wamp, wamp_d), (wdmp, wdmp_d), (wdiff, wdiff_d),
                           (wg1a, wg1a_d), (wg1b, wg1b_d), (wg2, wg2_d),
                           (bdiffT, bdiff_d), (ones1, ones_d), (bamp, bamp_d),
                           (bdmp, bdmp_d), (bg1, bg1_d), (bg2, bg2_d)]:
                nc.sync.dma_start(out=t_[:], in_=d_[:])
            nc.sync.dma_start(out=idxP_t[:, cutP:], in_=idxP_d[:, cutP:])
            nc.sync.dma_start(out=idxR_t[:, cutR:], in_=idxR_d[:, cutR:])

            self_q = [0]
            gmi = [0, 0]       # group cursor per pass
            ci = [0, 0]        # chunk cursor per pass

            def scatter_sb(pi, which, idx_t, offA_t, offB_t, src_d, cmeta,
                           gmeta, ps, wp, sp):
                for _ in range(REG):
                    _, r, n_in_group, base_slot = gmeta[gmi[pi]]
                    gmi[pi] += 1
                    if n_in_group == 0:
                        continue
                    xg = wp.tile([128, n_in_group, F2], dt.bfloat16,
                                 tag=f"xg{which}", bufs=6)
                    g0 = 0
                    sizes = _call_sizes(n_in_group)
                    for si, gn in enumerate(sizes):
                        # last call of the group: drop the 16 reserved pad
                        # slots (stale xg lanes are zeroed by off=200 S rows)
                        nidx = gn * 128 - (16 if si == len(sizes) - 1 else 0)
                        nc.gpsimd.dma_gather(
                            xg[:, g0:g0 + gn, :],
                            src_d[r * PREGPAD:(r + 1) * PREGPAD, :],
                            idx_t[:, (base_slot + g0 * 128) // 16:
                                     (base_slot + g0 * 128 + nidx) // 16],
                            nidx, nidx, F2,
                            queue_num=self_q[0] % 4)
                        self_q[0] += 1
                        g0 += gn
                    nb_done = 0
                    while nb_done < n_in_group:
                        nb = min(CB, n_in_group - nb_done)
                        SA = sp.tile([128, CB, W], dt.bfloat16, tag="SA", bufs=6)
                        SB = sp.tile([128, CB, W], dt.bfloat16, tag="SB", bufs=6)
                        c0c = ci[pi] + nb_done
                        nc.vector.tensor_tensor(
                            out=SA[:, :nb, :],
                            in0=iota_t[:, :nb * W].rearrange("p (c w) -> p c w", w=W),
                            in1=offA_t[:, c0c:c0c + nb].to_broadcast([128, nb, W]),
                            op=mybir.AluOpType.is_equal,
                        )
                        nc.vector.tensor_tensor(
                            out=SB[:, :nb, :],
                            in0=iota_t[:, :nb * W].rearrange("p (c w) -> p c w", w=W),
                            in1=offB_t[:, c0c:c0c + nb].to_broadcast([128, nb, W]),
                            op=mybir.AluOpType.is_equal,
                        )
                        for j in range(nb):
                            _, _, bl, st, sp_ = cmeta[c0c + j]
                            nc.tensor.matmul(
                                out=ps[:, bl * W:(bl + 1) * W],
                                lhsT=xg[:, nb_done + j, 0:F],
                                rhs=SA[:, j, :],
                                start=bool(st), stop=False,
                            )
                            nc.tensor.matmul(
                                out=ps[:, bl * W:(bl + 1) * W],
                                lhsT=xg[:, nb_done + j, F:F2],
                                rhs=SB[:, j, :],
                                start=False, stop=bool(sp_),
                            )
                        nb_done += nb
                    ci[pi] += n_in_group

            with (
                tc.tile_pool(name="slices", bufs=3) as slp,
                tc.tile_pool(name="gwork", bufs=3) as wp,
                tc.tile_pool(name="stiles", bufs=4) as sp,
                tc.tile_pool(name="ptq", bufs=2) as ptp,
                tc.tile_pool(name="dwork", bufs=2) as dwp,
                tc.tile_pool(name="psacc", bufs=1, space="PSUM") as psa,
                tc.tile_pool(name="psdense", bufs=3, space="PSUM") as psd,
            ):
                for sb in range(NSB):
                    nblk = min(SBW, NB - sb * SBW)
                    cw = nblk * W
                    c0 = sb * SBW * W
                    dis_sl = slp.tile([1, cw], dt.bfloat16, tag="dis")
                    nc.sync.dma_start(out=dis_sl[:], in_=dcnt_d[0:1, c0:c0 + cw])
                    cnt_sl = slp.tile([1, cw], dt.bfloat16, tag="cnt")
                    nc.sync.dma_start(out=cnt_sl[:], in_=dcnt_d[1:2, c0:c0 + cw])
                    xdisT_sl = slp.tile([128, cw], dt.bfloat16, tag="xdisTs")
                    nc.sync.dma_start(out=xdisT_sl[:], in_=xdisT_d[:, c0:c0 + cw])
                    xlocT_sl = slp.tile([128, cw], dt.bfloat16, tag="xlocTs")
                    nc.sync.dma_start(out=xlocT_sl[:], in_=xlocT_d[:, c0:c0 + cw])

                    # ---- P scatter ----
                    psP = psa.tile([128, cw], dt.float32, tag="psP")
                    scatter_sb(0, "P", idxP_t, offPA_t, offPB_t, xdis_d, cmP,
                               gmP, psP, wp, sp)
                    PT_t = ptp.tile([128, cw], dt.bfloat16, tag="PT")
                    nc.vector.tensor_tensor(out=PT_t[:], in0=psP[:],
                                            in1=xdisT_sl[:],
                                            op=mybir.AluOpType.add)

                    # ---- dense part 1 (PT-only: h_align, relu-h) ----
                    hAs, hDs = [], []
                    for j in range(0, cw, 512):
                        jw = min(512, cw - j)
                        dis_row = dis_sl[:, j:j + jw]
                        dsb_ps = psd.tile([128, jw], dt.float32, tag="ps")
                        nc.tensor.matmul(out=dsb_ps[:], lhsT=ones1[:], rhs=dis_row,
                                         start=True, stop=True)
                        PTs = dwp.tile([128, jw], dt.bfloat16, tag="PTs")
                        nc.vector.tensor_tensor(out=PTs[:], in0=dsb_ps[:],
                                                in1=PT_t[:, j:j + jw],
                                                op=mybir.AluOpType.mult)
                        hA_ps = psd.tile([128, jw], dt.float32, tag="ps")
                        nc.tensor.matmul(out=hA_ps[:], lhsT=wamp[:], rhs=PTs[:],
                                         start=True, stop=True)
                        hA = dwp.tile([128, jw], dt.bfloat16, tag="hA", bufs=3)
                        nc.scalar.activation(hA[:], hA_ps[:], AF.Identity, bias=bamp[:])
                        hD_ps = psd.tile([128, jw], dt.float32, tag="ps")
                        nc.tensor.matmul(out=hD_ps[:], lhsT=wdmp[:], rhs=PTs[:],
                                         start=True, stop=True)
                        hD = dwp.tile([128, jw], dt.bfloat16, tag="hD", bufs=3)
                        nc.scalar.activation(hD[:], hD_ps[:], AF.Relu, bias=bdmp[:])
                        hAs.append(hA)
                        hDs.append(hD)

                    # ---- R scatter ----
                    psR = psa.tile([128, cw], dt.float32, tag="psR")
                    scatter_sb(1, "R", idxR_t, offRA_t, offRB_t, xraw_d, cmR,
                               gmR, psR, wp, sp)
                    QT_t = ptp.tile([128, cw], dt.bfloat16, tag="QT")
                    nc.vector.tensor_tensor(out=QT_t[:], in0=psR[:],
                                            in1=xlocT_sl[:],
                                            op=mybir.AluOpType.mult)

                    # ---- dense part 2 (needs QT) ----
                    out_sb = dwp.tile([128, cw], dt.bfloat16, tag="outsb")
                    for j in range(0, cw, 512):
                        jw = min(512, cw - j)
                        cnt_row = cnt_sl[:, j:j + jw]
                        hA = hAs[j // 512]
                        hD = hDs[j // 512]
                        hC_ps = psd.tile([128, jw], dt.float32, tag="ps")
                        nc.tensor.matmul(out=hC_ps[:], lhsT=wdiff[:],
                                         rhs=QT_t[:, j:j + jw], start=True, stop=False)
                        nc.tensor.matmul(out=hC_ps[:], lhsT=bdiffT[:], rhs=cnt_row,
                                         start=False, stop=True)
                        hdiv = dwp.tile([128, jw], dt.bfloat16, tag="hdiv")
                        nc.vector.tensor_tensor(out=hdiv[:], in0=hC_ps[:], in1=hD[:],
                                                op=mybir.AluOpType.add)
                        pre_ps = psd.tile([128, jw], dt.float32, tag="ps")
                        nc.tensor.matmul(out=pre_ps[:], lhsT=wg1a[:], rhs=hA[:],
                                         start=True, stop=False)
                        nc.tensor.matmul(out=pre_ps[:], lhsT=wg1b[:], rhs=hdiv[:],
                                         start=False, stop=True)
                        pre = dwp.tile([128, jw], dt.bfloat16, tag="pre")
                        nc.scalar.activation(pre[:], pre_ps[:], AF.Relu, bias=bg1[:])
                        al_ps = psd.tile([1, jw], dt.float32, tag="al", bufs=1)
                        nc.tensor.matmul(out=al_ps[:], lhsT=wg2[:], rhs=pre[:],
                                         start=True, stop=True)
                        alpha = dwp.tile([1, jw], dt.bfloat16, tag="alpha")
                        nc.scalar.activation(alpha[:], al_ps[:], AF.Sigmoid, bias=bg2[:])
                        ab_ps = psd.tile([128, jw], dt.float32, tag="ps")
                        nc.tensor.matmul(out=ab_ps[:], lhsT=ones1[:], rhs=alpha[:],
                                         start=True, stop=True)
                        dif = dwp.tile([128, jw], dt.bfloat16, tag="dif")
                        nc.vector.tensor_tensor(out=dif[:], in0=hA[:], in1=hdiv[:],
                                                op=mybir.AluOpType.subtract)
                        prod = dwp.tile([128, jw], dt.bfloat16, tag="prod")
                        nc.vector.tensor_tensor(out=prod[:], in0=ab_ps[:], in1=dif[:],
                                                op=mybir.AluOpType.mult)
                        nc.vector.tensor_tensor(out=out_sb[:, j:j + jw], in0=hdiv[:],
                                                in1=prod[:], op=mybir.AluOpType.add)
                    nc.scalar.dma_start(out=out_d[:, c0:c0 + cw], in_=out_sb[:])

    nc.finalize()
    # DMASW completion-sem lanes rotate mod 8 in FINAL order; pair lane (n%8)
    # with queue (n%8)%4.
    n = 0
    for block in nc.m.functions[0].blocks:
        for inst in block.instructions:
            if inst.__class__.__name__ == "InstDMAGatherAnt":
                inst.queue_num = (n % 8) % 4
                n += 1
    return nc


# ----------------------------------------------------------------------------
# entry point
# ----------------------------------------------------------------------------

def _install_ntff_shim():
    import types
    if "antenv.axon_hooks" in sys.modules:
        return
    try:
        import antenv  # noqa: F401
        from trn_agent_boot.trn_boot import _ntff_profile_via_ctypes
        mod = types.ModuleType("antenv.axon_hooks")
        mod._hook = None
        mod.set_axon_ntff_profile_hook = lambda h: setattr(mod, "_hook", h)
        mod.get_axon_ntff_profile_hook = lambda: mod._hook
        sys.modules["antenv.axon_hooks"] = mod
        setattr(sys.modules["antenv"], "axon_hooks", mod)
        mod.set_axon_ntff_profile_hook(
            _ntff_profile_via_ctypes("/opt/axon/libaxon_pjrt.so"))
    except Exception:
        pass


def _prep(x, edge_index, W_amp, b_amp, W_dmp, b_dmp, W_diff, b_diff, Wg1, bg1,
          Wg2, bg2):
    """Host planning + per-core input maps. Returns (nc, in_maps)."""
    x = np.asarray(x, np.float32)
    edge_index = np.asarray(edge_index)
    row = edge_index[0].astype(np.int64)
    col = edge_index[1].astype(np.int64)

    deg = 1.0 + np.bincount(col, minlength=N_NODES).astype(np.float64)
    dis = (1.0 / np.sqrt(deg)).astype(np.float32)
    cnt_row = np.bincount(row, minlength=N_NODES).astype(np.float32)

    coreP = col // SH          # P-pass: scatter dest = col, gather src = row
    coreR = row // SH          # R-pass: scatter dest = row, gather src = col

    plans = []                 # per core: (P pairing, R pairing)
    sbP, srP, sbR, srR = [], [], [], []
    for k in range(N_CORES):
        mP = coreP == k
        mR = coreR == k
        pP = _pair_stream(row[mP], col[mP] - k * SH)
        pR = _pair_stream(col[mR], row[mR] - k * SH)
        jP = _assign_regions(pP[2])
        jR = _assign_regions(pR[2])
        plans.append((pP, jP, pR, jR))
        sbP.append(pP[2]); srP.append(jP // PREG)
        sbR.append(pR[2]); srR.append(jR // PREG)

    nchP = _plan(sbP, srP)
    nchR = _plan(sbR, srR)
    gposP, gbaseP, cmP, gmP = _group_meta(nchP)
    gposR, gbaseR, cmR, gmR = _group_meta(nchR)
    lenP, lenR = int(gbaseP[-1]), int(gbaseR[-1])

    key = (lenP, lenR, tuple(nchP.ravel()), tuple(nchR.ravel()))
    if key not in _CACHE:
        _CACHE[key] = _build_graph(cmP, gmP, cmR, gmR, lenP, lenR)
    nc = _CACHE[key]

    xdis_full = (x * dis[:, None]).astype(bf16)
    xraw_full = x.astype(bf16)

    iota = np.ascontiguousarray(np.tile(np.arange(W), (128, CB)).astype(bf16))
    wamp_h = np.ascontiguousarray(np.asarray(W_amp).astype(bf16))
    wdmp_h = np.ascontiguousarray(np.asarray(W_dmp).astype(bf16))
    wdiff_h = np.ascontiguousarray(np.asarray(W_diff).astype(bf16))
    wg1a_h = np.ascontiguousarray(np.asarray(Wg1)[:128].astype(bf16))
    wg1b_h = np.ascontiguousarray(np.asarray(Wg1)[128:].astype(bf16))
    wg2_h = np.ascontiguousarray(np.asarray(Wg2).astype(bf16))
    ones_h = np.ones((1, 128), bf16)
    bdiff_h = np.ascontiguousarray(np.asarray(b_diff, np.float32).reshape(1, 128).astype(bf16))
    bamp_h = np.ascontiguousarray(np.asarray(b_amp, np.float32).reshape(128, 1))
    bdmp_h = np.ascontiguousarray(np.asarray(b_dmp, np.float32).reshape(128, 1))
    bg1_h = np.ascontiguousarray(np.asarray(bg1, np.float32).reshape(128, 1))
    bg2_h = np.ascontiguousarray(np.asarray(bg2, np.float32).reshape(1, 1))

    in_maps = []
    for k in range(N_CORES):
        (ruP, rvP, rbP, raP, roP), jP, (ruR, rvR, rbR, raR, roR), jR = plans[k]
        idxsP, offPA, offPB = _stream(rbP, jP // PREG, jP % PREG, raP, roP,
                                      gposP, gbaseP)
        idxsR, offRA, offRB = _stream(rbR, jR // PREG, jR % PREG, raR, roR,
                                      gposR, gbaseR)
        tabP = _pair_table(xdis_full, ruP, rvP, jP)
        tabR = _pair_table(xraw_full, ruR, rvR, jR)
        lo, hi = k * SH, (k + 1) * SH
        xlocT = np.zeros((128, NBW), bf16)
        xlocT[:, :SH] = xraw_full[lo:hi].T
        xdisT = np.zeros((128, NBW), bf16)
        xdisT[:, :SH] = xdis_full[lo:hi].T
        dcnt = np.zeros((2, NBW), bf16)
        dcnt[0, :SH] = dis[lo:hi].astype(bf16)
        dcnt[1, :SH] = cnt_row[lo:hi].astype(bf16)
        in_maps.append({
            "xdis": tabP, "xraw": tabR,
            "idxP": _wrap_idx(idxsP), "idxR": _wrap_idx(idxsR),
            "offPA": _wrap_off(offPA), "offPB": _wrap_off(offPB),
            "offRA": _wrap_off(offRA), "offRB": _wrap_off(offRB),
            "xlocT": xlocT, "xdisT": xdisT, "dcnt": dcnt,
            "iota": iota, "wamp": wamp_h, "wdmp": wdmp_h, "wdiff": wdiff_h,
            "wg1a": wg1a_h, "wg1b": wg1b_h, "wg2": wg2_h, "bdiffT": bdiff_h,
            "ones1": ones_h, "bamp": bamp_h, "bdmp": bdmp_h, "bg1": bg1_h,
            "bg2": bg2_h,
        })
    return nc, in_maps


def kernel(x, edge_index, W_amp, b_amp, W_dmp, b_dmp, W_diff, b_diff, Wg1, bg1,
           Wg2, bg2, _trace=False):
    global _LAST_EXEC_NS
    _install_ntff_shim()
    from concourse.bass_utils import run_bass_kernel_spmd

    nc, in_maps = _prep(x, edge_index, W_amp, b_amp, W_dmp, b_dmp, W_diff,
                        b_diff, Wg1, bg1, Wg2, bg2)

    res = None
    if _trace:
        try:
            res = run_bass_kernel_spmd(nc, in_maps, core_ids=list(range(N_CORES)),
                                       trace=True)
            _LAST_EXEC_NS = res.exec_time_ns
        except Exception as e:
            print("trace run failed, falling back:", e, file=sys.stderr)
            res = None
    if res is None:
        res = run_bass_kernel_spmd(nc, in_maps, core_ids=list(range(N_CORES)))

    out = np.concatenate(
        [np.asarray(res.results[k]["out"])[:, :SH].T.astype(np.float32)
         for k in range(N_CORES)], axis=0)
    return np.ascontiguousarray(out)
